# revision 31
# baseline (speedup 1.0000x reference)
"""Single-head attention (B=4, S=2048, E=1024, fp32) on 8 trn2 NeuronCores.

Sharding: (batch, key-half) -> 8 shards. Core c handles batch c//2 and half
h = c%2: keys/values AND queries [h*1024, (h+1)*1024) of x[b]. Per core:
  - Q/K/V projections for its own 1024 rows only (Wq/Wk applied without the
    1/sqrt(E) scale; outputs quantized straight to fp8e4m3, which needs the
    natural value range -- the scale is applied later inside the exp).
  - Q^T halves are exchanged within each core pair via a DRAM AllGather
    (replica_groups [[0,1],[2,3],[4,5],[6,7]]), landing in global query
    order; the exchange overlaps the K and V projections.
  - scores^T = K^T-stationary x Q^T-moving in fp8 DoubleRow perf mode
    (2 f-tiles contracted per matmul at 0.5 cycles/row), exp via the scalar
    engine (scale=1/32) into a bf16 pt tile kept resident in SBUF.
  - V carries an extra ones column (padded tile), so the O matmul
    O_h = pt^T @ [V_h | 1] yields the softmax denominators rs_h as column
    1024 -- no separate rowsum pass on the PE.
  - Outputs: unnormalized O_h (bf16) + rs_h; the host combines
    out = (O_0 + O_1) / (rs_0 + rs_1) + bv  (the V bias commutes with the
    softmax average, so it is added once on the host).

Matmul dtype/perf notes (measured on HW):
  - bf16 matmuls hit the 215ns/512-col streaming floor ONLY when the
    stationary operand is reused across consecutive matmuls; swapping the
    stationary every matmul exposes ~45ns of FWL LDWEIGHTS. All loops keep
    each stationary for 2-4 consecutive matmuls.
  - SBUF tiles must keep 16-byte-aligned free-dim rows; the V tile is
    padded to 1040 columns for this.
  - A small burst of warmup matmuls on a memzero'd scratch tile (no DMA
    dependency) runs during the initial input load to cover the PE's
    activity-based clock ramp. Keep the burst SHORT (~22): long continuous
    early bursts trip the sustained-power P0 downclock (PE drops to 2.0GHz
    for the whole kernel, 215 -> 258 ns/matmul). The same throttled state
    is entered by chip thermal history (back-to-back heavy runs); a few
    minutes of device idle restores the 2.4GHz state.

SBUF layout (per core):
  xt [128, 8e, 1024] bf16  own 1024 rows of x[b]^T (moving + V stationary)
  w  [128, 8e, 128]  bf16  Wq^T / Wk^T stationary tiles (streamed)
  qt [128, 8f, 1024] fp8   own Q^T half; qt_g [128, 8f, 2048] gathered
  kt [128, 8f, 1024] fp8   K^T (scores stationary)
  wv [128, 8e, 1024] bf16  Wv^T (V-proj moving)
  v  [128, 8k, 1040] bf16  [V | 1 | pad] (O moving)
  pt [128, 8k, 2048] bf16  exp(scores^T) (O stationary)
"""

import numpy as np

P = 128


def _emit(nc, E=1024, S=2048, SK=1024):
    import concourse.mybir as mybir
    import concourse.tile as tile

    f32 = mybir.dt.float32
    f32r = mybir.dt.float32r
    bf16 = mybir.dt.bfloat16
    fp8 = mybir.dt.float8e4
    ACT = mybir.ActivationFunctionType

    ET = E // P     # e/f tiles (8)
    QT = S // P     # q tiles (16)
    KT = SK // P    # k tiles (8)
    NQC = S // 512  # q chunks (4)
    NKC = SK // 512  # k chunks (2)
    NFC = E // 512  # f chunks (2)

    xt8 = nc.dram_tensor("xt8", [ET, P, SK], bf16, kind="ExternalInput")
    wq8 = nc.dram_tensor("wq8", [ET, P, ET, P], bf16, kind="ExternalInput")
    wk8 = nc.dram_tensor("wk8", [ET, P, ET, P], bf16, kind="ExternalInput")
    wv8 = nc.dram_tensor("wv8", [ET, P, E], bf16, kind="ExternalInput")
    bq8 = nc.dram_tensor("bq8", [P, ET], f32, kind="ExternalInput")
    bk8 = nc.dram_tensor("bk8", [P, ET], f32, kind="ExternalInput")
    ones8 = nc.dram_tensor("ones8", [P, 8], bf16, kind="ExternalInput")
    ou = nc.dram_tensor("ou", [S, E], bf16, kind="ExternalOutput")
    rs = nc.dram_tensor("rs", [P, QT], f32, kind="ExternalOutput")

    groups = [[2 * i, 2 * i + 1] for i in range(4)]

    with tile.TileContext(nc) as tc:
        with (
            tc.tile_pool(name="dramp", bufs=1, space="DRAM") as dramp,
            tc.tile_pool(name="psum", bufs=6, space="PSUM") as psum,
            tc.tile_pool(name="small", bufs=1) as small,
            tc.tile_pool(name="persist", bufs=1) as pers,
            tc.tile_pool(name="obuf", bufs=3) as obp,
        ):
            wz_t = small.tile([P, 512], bf16, tag="wz")
            nc.scalar.memzero(wz_t[:])
            ones_t = small.tile([P, 8], bf16, tag="ones")
            nc.gpsimd.dma_start(ones_t[:], ones8[:])
            bq_t = small.tile([P, ET], f32, tag="bq")
            nc.sync.dma_start(bq_t[:], bq8[:])
            bk_t = small.tile([P, ET], f32, tag="bk")
            nc.sync.dma_start(bk_t[:], bk8[:])
            rs_sb = small.tile([P, QT], f32, tag="rssb")

            qt_g = pers.tile([P, ET, S], fp8, tag="qtg")
            kt_t = pers.tile([P, ET, SK], fp8, tag="kt")
            v_t = pers.tile([P, KT, E + 16], bf16, tag="v")

            warm_ps = psum.tile([1, 512], f32, tag="warm", bufs=1)

            with (
                tc.tile_pool(name="ph1", bufs=1) as ph1,
                tc.tile_pool(name="wstream", bufs=3) as wsp,
            ):
                xt_t = ph1.tile([P, ET, SK], bf16, tag="xt")
                wv_t = ph1.tile([P, ET, E], bf16, tag="wv")
                qt_t = ph1.tile([P, ET, SK], fp8, tag="qt")
                qh_d = dramp.tile([P, ET, SK], fp8, tag="qhd")
                qg_d = dramp.tile([2, P, ET, SK], fp8, tag="qgd")

                # ---- PE warmup (HAM clock ramp): zeroed scratch needs no
                # DMA, so warmups start at engine boot and span the input load
                for i in range(22):
                    nc.tensor.matmul(
                        warm_ps[:], wz_t[:, 0:1], wz_t[:],
                        start=True, stop=True,
                    )

                # ---- input DMA: weights on sync; xt spread over all three
                # rings, chunk-sized for early compute starts ----
                wq_rows = []
                w_t = wsp.tile([P, ET, P], bf16, tag="w", name="wq_f0")
                nc.sync.dma_start(w_t[:], wq8[0])
                wq_rows.append(w_t)
                xt_rings = [nc.gpsimd, nc.scalar, nc.sync]
                # e-major emission matches the Q-projection's consumption
                # order (e-outer loop), so arrivals never outrun demand
                for e in range(ET):
                    for kc in range(NKC):
                        xt_rings[e % 3].dma_start(
                            xt_t[:, e, kc * 512 : (kc + 1) * 512],
                            xt8[e, :, kc * 512 : (kc + 1) * 512],
                        )
                for f in range(1, ET):
                    w_t = wsp.tile([P, ET, P], bf16, tag="w", name=f"wq_f{f}")
                    nc.sync.dma_start(w_t[:], wq8[f])
                    wq_rows.append(w_t)
                wk_rows = []
                for f in range(ET):
                    w_t = wsp.tile([P, ET, P], bf16, tag="w", name=f"wk_f{f}")
                    nc.sync.dma_start(w_t[:], wk8[f])
                    wk_rows.append(w_t)
                for e in range(ET):
                    nc.scalar.dma_start(wv_t[:, e], wv8[e])

                # ---- Q projection, own query half only (the first SK
                # permuted columns = global queries [h*SK,(h+1)*SK)) ----
                for f in range(ET):
                    ps4 = [
                        psum.tile([P, 512], f32, tag="mm", name=f"q{f}_{qc}")
                        for qc in range(NKC)
                    ]
                    for e in range(ET):
                        for qc in range(NKC):
                            nc.tensor.matmul(
                                ps4[qc][:],
                                wq_rows[f][:, e],
                                xt_t[:, e, qc * 512 : (qc + 1) * 512],
                                start=(e == 0),
                                stop=(e == ET - 1),
                            )
                    for qc in range(NKC):
                        nc.scalar.add(
                            qt_t[:, f, qc * 512 : (qc + 1) * 512],
                            ps4[qc][:],
                            bq_t[:, f : f + 1],
                        )

                # stage own half to DRAM, pair-allgather into global order,
                # read back both halves; overlaps with the V projection
                nc.gpsimd.dma_start(qh_d[:], qt_t[:])
                nc.gpsimd.collective_compute(
                    "AllGather",
                    mybir.AluOpType.bypass,
                    replica_groups=groups,
                    ins=[qh_d[:]],
                    outs=[qg_d[:]],
                )
                for g in range(2):
                    nc.gpsimd.dma_start(
                        qt_g[:, :, g * SK : (g + 1) * SK], qg_d[g]
                    )

                # ---- K projection (key half = first SK columns of xt) ----
                for f in range(ET):
                    ps2 = [
                        psum.tile([P, 512], f32, tag="mm", name=f"k{f}_{kc}")
                        for kc in range(NKC)
                    ]
                    for e in range(ET):
                        for kc in range(NKC):
                            nc.tensor.matmul(
                                ps2[kc][:],
                                wk_rows[f][:, e],
                                xt_t[:, e, kc * 512 : (kc + 1) * 512],
                                start=(e == 0),
                                stop=(e == ET - 1),
                            )
                    for kc in range(NKC):
                        nc.scalar.add(
                            kt_t[:, f, kc * 512 : (kc + 1) * 512],
                            ps2[kc][:],
                            bk_t[:, f : f + 1],
                        )

                # ---- V projection: v[k, f] = sum_e xt[e, k] * wv[e, f] ----
                for kt in range(KT):
                    ps2 = [
                        psum.tile([P, 512], f32, tag="mm", name=f"v{kt}_{fc}")
                        for fc in range(NFC)
                    ]
                    for e in range(ET):
                        for fc in range(NFC):
                            nc.tensor.matmul(
                                ps2[fc][:],
                                xt_t[:, e, kt * P : (kt + 1) * P],
                                wv_t[:, e, fc * 512 : (fc + 1) * 512],
                                start=(e == 0),
                                stop=(e == ET - 1),
                            )
                    for fc in range(NFC):
                        nc.vector.tensor_copy(
                            v_t[:, kt, fc * 512 : (fc + 1) * 512], ps2[fc][:]
                        )
                    nc.vector.tensor_copy(v_t[:, kt, E : E + 1], ones_t[:, 0:1])

            with tc.tile_pool(name="ptp", bufs=1) as ptp:
                pt_t = ptp.tile([P, KT, S], bf16, tag="pt")

                # ---- scores^T (fp8 DoubleRow) + exp; rowsums in a second
                # pass so the PE never waits on the scalar exp ----
                DR = mybir.MatmulPerfMode.DoubleRow
                scale = float(1.0 / np.sqrt(np.float32(E)))
                for qh in range(2):
                    for kt in range(KT):
                        ps2 = [
                            psum.tile([P, 512], f32, tag="mm", name=f"s{kt}_{qc}")
                            for qc in range(2)
                        ]
                        for fp in range(ET // 2):
                            for qc in range(2):
                                col = qh * 1024 + qc * 512
                                nc.tensor.matmul(
                                    ps2[qc][:],
                                    kt_t[:, 2 * fp : 2 * fp + 2, kt * P : (kt + 1) * P],
                                    qt_g[:, 2 * fp : 2 * fp + 2, col : col + 512],
                                    start=(fp == 0),
                                    stop=(fp == ET // 2 - 1),
                                    perf_mode=DR,
                                )
                        for qc in range(2):
                            col = qh * 1024 + qc * 512
                            nc.scalar.activation(
                                pt_t[:, kt, col : col + 512], ps2[qc][:], ACT.Exp,
                                scale=scale,
                            )

                # ---- O = pt^T @ v, unnormalized; store bf16 ----
                CH = [(0, 512), (512, 768), (768, E + 1)]
                for qt in range(QT):
                    po = [
                        psum.tile([P, 512], f32, tag="mm", name=f"o{qt}_{fc}")
                        for fc in range(len(CH))
                    ]
                    for kt in range(KT):
                        for fc, (c0, c1) in enumerate(CH):
                            nc.tensor.matmul(
                                po[fc][:, : c1 - c0],
                                pt_t[:, kt, qt * P : (qt + 1) * P],
                                v_t[:, kt, c0:c1],
                                start=(kt == 0),
                                stop=(kt == KT - 1),
                            )
                    o_sb = obp.tile([P, E], bf16, tag="ob")
                    nc.vector.tensor_copy(o_sb[:, 0:512], po[0][:])
                    nc.sync.dma_start(
                        ou[qt * P : (qt + 1) * P, 0:512], o_sb[:, 0:512]
                    )
                    nc.vector.tensor_copy(o_sb[:, 512:768], po[1][:, :256])
                    nc.vector.tensor_copy(o_sb[:, 768:E], po[2][:, :256])
                    nc.vector.tensor_copy(
                        rs_sb[:, qt : qt + 1], po[2][:, 256:257]
                    )
                    nc.sync.dma_start(
                        ou[qt * P : (qt + 1) * P, 512:E], o_sb[:, 512:E]
                    )
                nc.sync.dma_start(rs[:], rs_sb[:])


_NC_CACHE = {}


def build_nc(E=1024, S=2048, SK=1024):
    key = (E, S, SK)
    if key in _NC_CACHE:
        return _NC_CACHE[key]
    import concourse.bacc as bacc

    nc = bacc.Bacc(None, target_bir_lowering=False)
    _emit(nc, E=E, S=S, SK=SK)
    nc.finalize()
    _NC_CACHE[key] = nc
    return nc


def _round_f32r(a):
    """Round fp32 to fp32r (tf32-like: 11 explicit mantissa bits, RNE)."""
    u = np.ascontiguousarray(a, np.float32).view(np.uint32)
    u = u + np.uint32(0x7FF) + ((u >> np.uint32(12)) & np.uint32(1))
    return (u & np.uint32(0xFFFFF000)).view(np.float32)


def make_in_maps(x, Wq, bq, Wk, bk, Wv, bv, E=1024, S=2048, SK=1024):
    """Host-side prep: per-core input dicts for run_bass_kernel_spmd."""
    import ml_dtypes

    bf16 = ml_dtypes.bfloat16
    ET = E // P
    scale = np.float32(1.0 / np.sqrt(np.float32(E)))
    x = np.asarray(x, np.float32)
    B = x.shape[0]
    n_half = S // SK

    def wtile(w):  # [f_tile, p(e), e_tile, c(f)] stationary blocks
        return np.ascontiguousarray(
            np.asarray(w, np.float32).reshape(ET, P, ET, P).transpose(0, 3, 2, 1)
        ).astype(bf16)

    wq8 = wtile(Wq)
    wk8 = wtile(Wk)
    # wv8[e, p, f] = Wv[f, e*128+p]
    wv8 = np.ascontiguousarray(
        np.asarray(Wv, np.float32).T.reshape(ET, P, E)
    ).astype(bf16)
    bq8 = np.ascontiguousarray(np.asarray(bq, np.float32).reshape(ET, P).T)
    bk8 = np.ascontiguousarray(np.asarray(bk, np.float32).reshape(ET, P).T)
    ones8 = np.ones((P, 8), bf16)

    in_maps = []
    for c in range(B * n_half):
        b, h = divmod(c, n_half)
        xt_half = x[b].T[:, h * SK : (h + 1) * SK]  # [E, SK]
        xt8 = np.ascontiguousarray(xt_half.reshape(ET, P, SK)).astype(bf16)
        in_maps.append(
            {
                "xt8": xt8,
                "wq8": wq8,
                "wk8": wk8,
                "wv8": wv8,
                "bq8": bq8,
                "bk8": bk8,
                "ones8": ones8,
            }
        )
    return in_maps


def kernel(x, Wq, bq, Wk, bk, Wv, bv):
    from concourse.bass_utils import run_bass_kernel_spmd

    E, S, SK = 1024, 2048, 1024
    x = np.asarray(x, np.float32)
    B = x.shape[0]
    n_half = S // SK
    nc = build_nc(E=E, S=S, SK=SK)
    in_maps = make_in_maps(x, Wq, bq, Wk, bk, Wv, bv, E=E, S=S, SK=SK)
    n_cores = len(in_maps)
    res = run_bass_kernel_spmd(nc, in_maps, list(range(n_cores)))

    bvf = np.asarray(bv, np.float32)
    out = np.empty((B, S, E), np.float32)
    for b in range(B):
        osum = None
        rsum = None
        for h in range(n_half):
            r = res.results[b * n_half + h]
            o_h = np.asarray(r["ou"]).astype(np.float32)
            rs_h = np.asarray(r["rs"]).astype(np.float32).T.reshape(S)
            osum = o_h if osum is None else osum + o_h
            rsum = rs_h if rsum is None else rsum + rs_h
        out[b] = osum / rsum[:, None] + bvf[None, :]
    return out
